# revision 27
# baseline (speedup 1.0000x reference)
"""Multi-head self-attention (b=4, L=2048, d=512, h=8) on 8 trn2 cores.

Sharding: data-parallel over batch (4) x tensor-parallel over heads (2 groups
of 4).  Core c handles batch c//2, heads [4*(c%2), 4*(c%2)+4).  Each core
returns a partial output (row-parallel Wo); the host sums the two partials per
batch and adds bo.

Device-side layout (all "transposed", so no on-device transposes are needed):
  xT   [512 d, 2048 q]   (host passes x[b].T, bf16)
  Q^T  [256 hd, 2048 q]  = WqT.T @ xT   (lhsT = WqT[d, hd] natural)
  K^T  same
  V    [2048 k, 256 hd]  (lhsT = xT chunks)  + ones column per head -> V_aug
  S^T  [128 k-tile, 512 q] = K^T_h.T-contraction(d_head=64)  -> PE 64-row mode,
       two heads of a pair run on independent half-arrays (T0 / T8)
  E^T  = exp(S^T * scale)  on ACT, one 1024-wide call per (k-tile, head-pair)
  O^T_aug [65, 512q] += V_aug_h.T @ E^T_h, split into two 64-deep contraction
       halves (stays in 64-row mode), giving O = P@V un-normalized plus the
       softmax denominator d[q] in row 64 (from the ones column).
  normalize: O = (O_a+O_b)[0:64] * broadcast(1/d) + bv  (DVE)
  outT [512 e, 2048 q] = WoT.T @ O_norm^T  (fp32 partial, host sums + adds bo)
"""

import numpy as np
import ml_dtypes

import concourse.bass as bass
import concourse.bacc as bacc
import concourse.tile as tile
import concourse.mybir as mybir
from concourse.bass_utils import run_bass_kernel_spmd

F32 = mybir.dt.float32
BF16 = mybir.dt.bfloat16

B, L, D = 4, 2048, 512
NH, DH = 8, 64
HG = 2                 # head groups (tensor parallel)
GH = NH // HG          # 4 heads per group
EG = GH * DH           # 256 columns per group
SCALE = 1.0 / float(np.sqrt(DH))
P = 128
KT = L // P            # 16 k-tiles
QC = L // 512          # 4 q-chunks of 512
DC = D // P            # 4 d-chunks

_ts = bass.ts


def _body(tc):
    nc = tc.nc
    xT = nc.dram_tensor("xT", [D, L], BF16, kind="ExternalInput")
    wqT = nc.dram_tensor("wqT", [D, EG], BF16, kind="ExternalInput")
    wkT = nc.dram_tensor("wkT", [D, EG], BF16, kind="ExternalInput")
    wvT = nc.dram_tensor("wvT", [D, EG], BF16, kind="ExternalInput")
    woT = nc.dram_tensor("woT", [EG, D], BF16, kind="ExternalInput")
    bqv = nc.dram_tensor("bq", [EG], F32, kind="ExternalInput")
    bkv = nc.dram_tensor("bk", [EG], F32, kind="ExternalInput")
    bvv = nc.dram_tensor("bv", [EG], F32, kind="ExternalInput")
    outT = nc.dram_tensor("outT", [D, L], F32, kind="ExternalOutput")

    add = mybir.AluOpType.add
    Exp = mybir.ActivationFunctionType.Exp

    with (
        tc.tile_pool(name="const", bufs=1) as const,
        # one PSUM pool for the whole kernel so the phases can overlap:
        #   tag "s": score tiles [128,2,512] (2 banks) x2  -> 4 banks
        #   tag "o": PV accumulators [65,512] x2           -> 2 banks
        #   tag "u": proj / out-proj psum [128,512] x2     -> 2 banks
        tc.tile_pool(name="ps", bufs=2, space="PSUM") as ps,
        tc.tile_pool(name="ew", bufs=3) as ew,
        tc.tile_pool(name="nw", bufs=2) as nw,
        tc.tile_pool(name="ow", bufs=3) as ow,
        tc.tile_pool(name="dr", bufs=2, space="DRAM") as dr,
    ):
        # ---- PE warm-up burst (runs while input DMAs are in flight) -------
        # The PE HAM up-clocks 1.2->2.4 GHz only after ~3.4us of sustained
        # matmul activity; issue throwaway matmuls so the real work runs warm.
        zwu = const.tile([P, 512], BF16)
        nc.vector.memset(zwu[:], 0.0)
        pwu = ps.tile([P, 512], F32, tag="u", name="warmup_ps")
        for _ in range(24):
            nc.tensor.matmul(pwu[:], zwu[:, 0:P], zwu[:], start=True, stop=True)

        # ---- load inputs --------------------------------------------------
        xT_sb = const.tile([P, DC, L], BF16)
        xT_r = xT[:, :].rearrange("(c p) k -> p c k", p=P)
        for c in range(DC):
            nc.sync.dma_start(out=xT_sb[:, c, :], in_=xT_r[:, c, :])

        wq_sb = const.tile([P, DC, EG], BF16)
        wk_sb = const.tile([P, DC, EG], BF16)
        wv_sb = const.tile([P, DC, EG], BF16)
        for w_sb, w_dr in ((wq_sb, wqT), (wk_sb, wkT), (wv_sb, wvT)):
            nc.sync.dma_start(
                out=w_sb[:], in_=w_dr[:, :].rearrange("(c p) e -> p c e", p=P))
        wo_sb = const.tile([P, HG, D], BF16)
        nc.sync.dma_start(
            out=wo_sb[:], in_=woT[:, :].rearrange("(c p) e -> p c e", p=P))

        bq_sb = const.tile([P, HG], F32)
        bk_sb = const.tile([P, HG], F32)
        nc.sync.dma_start(out=bq_sb[:], in_=bqv[:].rearrange("(t p) -> p t", p=P))
        nc.sync.dma_start(out=bk_sb[:], in_=bkv[:].rearrange("(t p) -> p t", p=P))
        # bv per-head: partition = dim-in-head, column = head
        # (bias is folded in after PV:  O/d + bv  — per-partition there)
        bv_sb = const.tile([DH, GH], F32)
        nc.sync.dma_start(out=bv_sb[:], in_=bvv[:].rearrange("(h p) -> p h", p=DH))

        # persistent activations
        qt_sb = const.tile([P, HG, L], BF16)     # Q^T  rows: head 2t + (r//64)
        kt_sb = const.tile([P, HG, L], BF16)     # K^T
        va_sb = const.tile([P, KT, GH, DH + 1], BF16)  # V + ones col, per k-tile
        on_sb = const.tile([P, HG, L], BF16)     # normalized O^T (attn output)

        nc.vector.memset(va_sb[:, :, :, DH:DH + 1], 1.0)

        # ---- projections (128-row PE mode) --------------------------------
        def qk_proj(t):
            for n in range(QC):
                for w_sb, dst, b_sb in (
                    (wq_sb, qt_sb, bq_sb),
                    (wk_sb, kt_sb, bk_sb),
                ):
                    psq = ps.tile([P, 512], F32, tag="u", name=f"qk_ps_{t}_{n}")
                    for c in range(DC):
                        nc.tensor.matmul(
                            psq[:],
                            w_sb[:, c, _ts(t, P)],
                            xT_sb[:, c, _ts(n, 512)],
                            start=(c == 0), stop=(c == DC - 1),
                        )
                    nc.vector.tensor_scalar(
                        out=dst[:, t, _ts(n, 512)], in0=psq[:],
                        scalar1=b_sb[:, t:t + 1], scalar2=None, op0=add,
                    )

        qk_proj(0)                   # heads 0,1 first so attention can start
        for i in range(KT):          # V (natural layout, k on partitions)
            psv = ps.tile([P, EG], F32, tag="u", name=f"v_ps_{i}")
            for c in range(DC):
                nc.tensor.matmul(
                    psv[:],
                    xT_sb[:, c, _ts(i, P)],
                    wv_sb[:, c, :],
                    start=(c == 0), stop=(c == DC - 1),
                )
            nc.vector.tensor_copy(
                out=va_sb[:, i, :, 0:DH],
                in_=psv[:].rearrange("p (h d) -> p h d", d=DH),
            )
        qk_proj(1)

        # ---- attention (64-row PE mode) + interleaved output projection ---
        for n in range(QC):          # q chunk of 512
            for pr in range(HG):     # head pair (heads 2pr, 2pr+1)
                o_t = [ps.tile([DH + 1, 512], F32, tag="o", bufs=2,
                               name=f"o_{pr}_{n}_{j}")
                       for j in range(2)]
                for i in range(KT):
                    s = ps.tile([P, 2, 512], F32, tag="s", bufs=2,
                                name=f"s_{pr}_{n}_{i}")
                    for j in range(2):
                        nc.tensor.matmul(
                            s[:, j, :],
                            kt_sb[_ts(j, DH), pr, _ts(i, P)],
                            qt_sb[_ts(j, DH), pr, _ts(n, 512)],
                            start=True, stop=True,
                        )
                    et = ew.tile([P, 2, 512], BF16, tag="et", bufs=4)
                    nc.scalar.activation(et[:], s[:], Exp, scale=SCALE)
                    for j in range(2):
                        nc.tensor.matmul(
                            o_t[j][:],
                            va_sb[:, i, 2 * pr + j, :],
                            et[:, j, :],
                            start=(i == 0), stop=(i == KT - 1),
                        )
                for j in range(2):
                    h = 2 * pr + j
                    # evacuate PSUM immediately so the banks recycle for the
                    # next block; normalize lazily from SBUF
                    ocp = nw.tile([DH + 1, 512], F32, tag="ocp")
                    nc.vector.tensor_copy(ocp[:], o_t[j][:])
                    dsb = nw.tile([1, 512], F32, tag="dsb")
                    nc.vector.tensor_copy(dsb[:], ocp[DH:DH + 1, :])
                    r = nw.tile([1, 512], F32, tag="r")
                    nc.vector.reciprocal_approx_fast(r[:], dsb[:])
                    # broadcast r across 64 partitions on GPSIMD
                    rb = nw.tile([DH, 512], F32, tag="rb")
                    nc.gpsimd.partition_broadcast(rb[:], r[:])
                    tmp = nw.tile([DH, 512], F32, tag="tmp")
                    nc.vector.tensor_mul(tmp[:], ocp[0:DH, :], rb[:])
                    nc.vector.tensor_scalar(
                        out=on_sb[_ts(j, DH), pr, _ts(n, 512)],
                        in0=tmp[:], scalar1=bv_sb[:, h:h + 1],
                        scalar2=None, op0=add,
                    )
            # output projection for this q chunk (128-row mode)
            for m in range(4):       # e tile
                pso = ps.tile([P, 512], F32, tag="u", name=f"o_ps_{m}_{n}")
                for c in range(HG):
                    nc.tensor.matmul(
                        pso[:],
                        wo_sb[:, c, _ts(m, P)],
                        on_sb[:, c, _ts(n, 512)],
                        start=(c == 0), stop=(c == HG - 1),
                    )
                st = ow.tile([P, 512], F32, tag="o_st")
                nc.vector.tensor_copy(st[:], pso[:])
                nc.sync.dma_start(
                    out=outT[:, :][_ts(m, P), _ts(n, 512)], in_=st[:],
                )


_CACHE = {}


def _get_nc():
    if "nc" not in _CACHE:
        # Bacc (not raw Bass): its compile() pipeline legalizes semaphore
        # waits (TRN2 allows at most one wait per instruction).
        nc = bacc.Bacc(None, target_bir_lowering=False)
        with tile.TileContext(nc) as tc:
            _body(tc)
        nc.finalize()
        _CACHE["nc"] = nc
    return _CACHE["nc"]


def make_in_maps(x, Wq, bq, Wk, bk, Wv, bv, Wo):
    bf = ml_dtypes.bfloat16
    in_maps = []
    for c in range(8):
        b, g = c // 2, c % 2
        es = slice(g * EG, (g + 1) * EG)
        in_maps.append({
            "xT": np.ascontiguousarray(np.asarray(x)[b].T).astype(bf),
            "wqT": np.ascontiguousarray(np.asarray(Wq)[es, :].T).astype(bf),
            "wkT": np.ascontiguousarray(np.asarray(Wk)[es, :].T).astype(bf),
            "wvT": np.ascontiguousarray(np.asarray(Wv)[es, :].T).astype(bf),
            "woT": np.ascontiguousarray(np.asarray(Wo)[:, es].T).astype(bf),
            "bq": np.ascontiguousarray(np.asarray(bq)[es]).astype(np.float32),
            "bk": np.ascontiguousarray(np.asarray(bk)[es]).astype(np.float32),
            "bv": np.ascontiguousarray(np.asarray(bv)[es]).astype(np.float32),
        })
    return in_maps


def gather_out(results, bo):
    bo = np.asarray(bo, dtype=np.float32)
    out = np.empty((B, L, D), np.float32)
    for b in range(B):
        out[b] = (results[2 * b]["outT"].T
                  + results[2 * b + 1]["outT"].T
                  + bo[None, :])
    return out


def kernel(x, Wq, bq, Wk, bk, Wv, bv, Wo, bo, **kwargs):
    nc = _get_nc()
    in_maps = make_in_maps(x, Wq, bq, Wk, bk, Wv, bv, Wo)
    res = run_bass_kernel_spmd(nc, in_maps, list(range(8)))
    return gather_out(res.results, bo)
